# revision 6
# baseline (speedup 1.0000x reference)
"""
CSAM (channel self-attention) Trainium2 Bass kernel, v2.

Computation (per batch b):
    q = x[b].reshape(C, N)                 # C=64, N=192*192=36864
    E = q @ q.T                            # [64, 64] channel gram
    A = softmax(rowmax(E) - E) over rows   # == softmax(-E) stabilized by rowmin
    out = A @ q
    res = x * (gamma * out) + x

Sharding: data-parallel over batch. 8 cores x 4 batches each; identical NEFF per
core on its own batch slice (no collectives).

v2 design (vs v1): PE was the bottleneck (transpose-mode MMs don't warm the HAM
clock; avg MM 269ns). Changes:
 - layout p = h*64 + c (x[b].rearrange("c (h j) -> (h c) j")): each partition is
   a contiguous DRAM run; transposed chunks get dense per-h channel halves, so
   energy matmuls need no parity reorder and W = blockdiag(A^T, A^T).
 - loads cast fp32->bf16 during SWDGE DMA: no staging buffers, no cast pass.
 - transposes are regular matmuls against a bf16 identity (FWL + warm clock).
 - E accumulated in one PSUM region over 288 [128,64]x[128,64] matmuls.
 - epilogue res = (po + 1) * q16 (gamma folded into A) on DVE; PSUM->SBUF
   transpose copies on ACT; res stored fp32 via HWDGE.
"""

import sys

sys.path.insert(0, "/opt/trn_rl_repo")

import numpy as np

import concourse.bass as bass
import concourse.bacc as bacc
import concourse.tile as tile
from concourse import mybir
from concourse.bass_utils import run_bass_kernel_spmd
from concourse.masks import make_identity

N_CORES = 8
B_FULL, C, H, W = 32, 64, 192, 192
N = H * W                  # 36864
NH = N // 2                # 18432 flat free width
B_PER = B_FULL // N_CORES  # 4 batches per core
NCH = NH // 128            # 144 transpose chunks per batch
TG = 4                     # chunks per transpose group (one PSUM bank)
NTG = NCH // TG            # 36 groups per batch
OC = 512                   # out-matmul chunk (one PSUM bank)
NOC = NH // OC             # 36 out chunks per batch
RQ = 4                     # res quarters per batch
RW = NH // RQ              # 4608 store width
NLD = 2                    # load chunks per batch

f32 = mybir.dt.float32
bf16 = mybir.dt.bfloat16

_CACHED_NC = None


def _build():
    nc = bacc.Bacc("TRN2", target_bir_lowering=False, debug=False)
    x_d = nc.dram_tensor("x", [B_PER, C, N], f32, kind="ExternalInput").ap()
    g_d = nc.dram_tensor("gamma", [1], f32, kind="ExternalInput").ap()
    o_d = nc.dram_tensor("out", [B_PER, C, N], f32, kind="ExternalOutput").ap()

    with tile.TileContext(nc) as tc:
        with (
            tc.tile_pool(name="const", bufs=1) as constp,
            tc.tile_pool(name="q16", bufs=3) as q16p,
            tc.tile_pool(name="qT", bufs=1) as qTp,
            tc.tile_pool(name="res", bufs=2) as resp,
            tc.tile_pool(name="sm", bufs=2) as smp,
            tc.tile_pool(name="w2", bufs=2) as w2p,
            tc.tile_pool(name="psT", bufs=3, space="PSUM") as psTp,
            tc.tile_pool(name="psE", bufs=2, space="PSUM") as psEp,
            tc.tile_pool(name="psO", bufs=2, space="PSUM") as psOp,
            tc.tile_pool(name="psA", bufs=1, space="PSUM") as psAp,
        ):
            ident16 = constp.tile([128, 128], bf16)
            make_identity(nc, ident16[:])
            g1 = constp.tile([1, 1], f32)
            nc.sync.dma_start(g1[:], g_d[None, :])
            gb = constp.tile([128, 1], f32)
            nc.gpsimd.partition_broadcast(gb[:], g1[:])

            def load(b):
                # p = h*64 + c; each partition one contiguous 73728B DRAM run.
                # DMA pairs stream order [h][c][j] <-> [p][j], so p = h*64+c.
                xb = x_d[b].rearrange("c (h j) -> h c j", h=2)
                q16 = q16p.tile([128, NH], bf16, tag="q16")
                w = NH // NLD
                for g in range(NLD):
                    nc.gpsimd.dma_start(
                        q16[:, g * w : (g + 1) * w], xb[:, :, g * w : (g + 1) * w]
                    )
                return q16

            def phase1(b, q16):
                """Transpose + energy accumulation; yields after each group."""
                qT = qTp.tile([128, NCH, 128], bf16, tag="qT")
                E_ps = psEp.tile([C, C], f32, tag="E")

                def emit_E(gi):
                    for i in range(TG):
                        t = gi * TG + i
                        for h in range(2):
                            lr = qT[:, t, 64 * h : 64 * h + 64]
                            nc.tensor.matmul(
                                E_ps[:],
                                lr,
                                lr,
                                start=(t == 0 and h == 0),
                                stop=(t == NCH - 1 and h == 1),
                            )

                for gi in range(NTG):
                    pq = psTp.tile([128, TG, 128], f32, tag="pq")
                    for i in range(TG):
                        t = gi * TG + i
                        nc.tensor.matmul(
                            pq[:, i, :],
                            q16[:, t * 128 : (t + 1) * 128],
                            ident16[:],
                            start=True,
                            stop=True,
                        )
                    nc.scalar.copy(qT[:, gi * TG : (gi + 1) * TG, :], pq[:])
                    # energy matmuls lag one group so the copy has time to land
                    if gi > 0:
                        emit_E(gi - 1)
                    yield gi
                emit_E(NTG - 1)
                phase1.E_ps = E_ps

            def softmax_W2(E_ps):
                E = smp.tile([C, C], f32, tag="E")
                nc.vector.tensor_copy(E[:], E_ps[:])
                m = smp.tile([C, 1], f32, tag="m")
                nc.vector.tensor_reduce(
                    m[:], E[:], axis=mybir.AxisListType.X, op=mybir.AluOpType.min
                )
                texp = smp.tile([C, C], f32, tag="texp")
                Z = smp.tile([C, 1], f32, tag="Z")
                nc.scalar.activation(
                    texp[:],
                    E[:],
                    mybir.ActivationFunctionType.Exp,
                    bias=m[:],
                    scale=-1.0,
                    accum_out=Z[:],
                )
                r = smp.tile([C, 1], f32, tag="r")
                nc.vector.reciprocal(r[:], Z[:])
                # fold gamma into A: epilogue becomes res = (out + 1) * x
                rg = smp.tile([C, 1], f32, tag="rg")
                nc.vector.tensor_tensor(
                    rg[:], r[:], gb[0:64, :], mybir.AluOpType.mult
                )
                A16 = smp.tile([C, C], bf16, tag="A16")
                nc.vector.tensor_scalar_mul(A16[:], texp[:], rg[:])
                # W2 = blockdiag(A^T, A^T) : po[h*64+c] = sum_d A[c,d] q[h*64+d]
                psA = psAp.tile([128, C], f32, tag="psA")
                nc.tensor.matmul(
                    psA[0:64, :], A16[:], ident16[0:64, 0:64], start=True, stop=True
                )
                nc.tensor.matmul(
                    psA[64:128, :], A16[:], ident16[0:64, 0:64], start=True, stop=True
                )
                W2 = w2p.tile([128, 128], bf16, tag="W2")
                nc.gpsimd.memset(W2[:], 0.0)
                nc.scalar.copy(W2[0:64, 0:64], psA[0:64, :])
                nc.vector.tensor_copy(W2[64:128, 64:128], psA[64:128, :])
                return W2

            def phase2_chunk(b, q16, W2, res_holder, k):
                ob = o_d[b].rearrange("c (h j) -> h c j", h=2)
                per = RW // OC  # out chunks per res quarter
                if k % per == 0:
                    res_holder[0] = resp.tile(
                        [128, RW], f32, tag="res", name="res"
                    )
                res = res_holder[0]
                po = psOp.tile([128, OC], f32, tag="po")
                nc.tensor.matmul(
                    po[:], W2[:], q16[:, k * OC : (k + 1) * OC], start=True, stop=True
                )
                off = (k % per) * OC
                nc.vector.scalar_tensor_tensor(
                    res[:, off : off + OC],
                    po[:],
                    1.0,
                    q16[:, k * OC : (k + 1) * OC],
                    mybir.AluOpType.add,
                    mybir.AluOpType.mult,
                )
                if k % per == per - 1:
                    quarter = k // per
                    nc.sync.dma_start(
                        ob[:, :, quarter * RW : (quarter + 1) * RW], res[:]
                    )

            prev = None
            for b in range(B_PER):
                q16 = load(b)
                for gi in phase1(b, q16):
                    if prev is not None:
                        phase2_chunk(*prev, gi)
                W2 = softmax_W2(phase1.E_ps)
                prev = (b, q16, W2, [None])
            for k in range(NOC):
                phase2_chunk(*prev, k)

    nc.compile()
    return nc


def _get_nc():
    global _CACHED_NC
    if _CACHED_NC is None:
        _CACHED_NC = _build()
    return _CACHED_NC


def kernel(x: np.ndarray, gamma: np.ndarray, _collect=None) -> np.ndarray:
    assert x.shape == (B_FULL, C, H, W) and x.dtype == np.float32
    nc = _get_nc()
    xr = np.ascontiguousarray(x.reshape(B_FULL, C, N), dtype=np.float32)
    gamma = np.ascontiguousarray(gamma, dtype=np.float32)
    in_maps = [
        {"x": xr[i * B_PER : (i + 1) * B_PER], "gamma": gamma}
        for i in range(N_CORES)
    ]
    r = run_bass_kernel_spmd(nc, in_maps, core_ids=list(range(N_CORES)))
    if _collect is not None:
        _collect.append(r)
    out = np.concatenate([r.results[i]["out"] for i in range(N_CORES)], axis=0)
    return out.reshape(B_FULL, C, H, W).astype(np.float32)
